# revision 12
# baseline (speedup 1.0000x reference)
"""Trainium2 Bass kernel for nn_PrototypicalHead.

Reference computation (per full problem):
    z = l2norm(spatial, axis=-1)            # [b, l, d]
    p = l2norm(prototypes, axis=-1)         # [c, j, d]
    sim = einsum('bld,cjd->blcj', z, p)
    pooled = max over l                     # [b, c, j]
    out = sum_j pooled * softplus(raw_w) + bias   # [b, c]

Full shapes: b=32, l=196, d=768, c=1000, j=10.

Sharding: 2-way data parallel over b x 4-way tensor parallel over c
(8 cores).  Per core: b_sh=16 (bl=3136 rows), c_sh=250 (cj=2500 cols).

Per-core device pipeline:
  - load z natural [bl, d] fp16; row sumsq (ACT Square+accum); sqrt;
    reciprocal; scale rows (DVE tensor_scalar); DMA-xbar-transpose to
    zT [d, bl] fp16 in SBUF.
  - pT [d, cj] fed pre-transposed from host (fp16, raw).  Prototype
    normalization is folded into the per-(c,j) weights:
        w' = softplus(raw_w) / ||p||
    which is legal because max over l commutes with the positive
    per-(c,j) scale.  Column sumsq of pT via ACT Square + ones-matmul.
  - main matmul: for each cj chunk of 128, psum[cj=128, bl-block 392]
    accumulated over 6 k-chunks (fp16, 1 cycle/row).  392 = 2*196 so
    each PSUM bank holds exactly two l-segments.
  - max over l: one DVE reduce_max per 4-bank psum tile with a 4D AP
    [128, 4, 2, 196] -> [128, 8].
  - j-sum: block-diagonal selector matmul.  S[cj, c_local] = w'[cj] at
    column (cj//10 - 128*(t//10)); 128 classes * 10 j = 10 cj-chunks
    align exactly with class-halves of 128.
  - bias add on ACT during PSUM->SBUF copy; single DMA out.

kernel() accepts FULL inputs and returns the FULL [32, 1000] fp32 output.
"""

import math

import numpy as np

import concourse.bass as bass
from concourse import bacc
import concourse.mybir as mybir
import concourse.tile as tile
from concourse.bass_utils import run_bass_kernel_spmd

F32 = mybir.dt.float32

# ---------------------------------------------------------------- config

class Cfg:
    def __init__(self, B=16, L=196, D=768, C=250, J=10, NB=2, NC=4,
                 fp=mybir.dt.float16, pb=4):
        self.B, self.L, self.D, self.C, self.J = B, L, D, C, J
        self.NB, self.NC = NB, NC          # mesh: batch shards x class shards
        self.fp = fp
        self.BL = B * L                    # rows of z per core (3136)
        self.KD = D // 128                 # k chunks (6)
        assert D % 128 == 0
        self.NZT = math.ceil(self.BL / 128)        # z row chunks (25)
        self.BLP = self.NZT * 128                  # padded rows (3200)
        self.NCJ = math.ceil(C * J / 128)          # cj chunks (20)
        self.CJP = self.NCJ * 128                  # padded cj (2560)
        assert self.CJP % J == 0
        self.CP = self.CJP // J                    # padded classes (256)
        assert self.CP % 128 == 0
        self.NCH = self.CP // 128                  # class halves (2)
        assert self.NCJ == self.NCH * J
        self.NBLK_W = 2 * L                        # psum block width (392)
        assert self.NBLK_W <= 512
        assert self.BL % self.NBLK_W == 0
        self.NBLK = self.BL // self.NBLK_W         # bl blocks (8)
        self.PB = min(pb, self.NBLK)               # blocks per psum tile (4)
        assert self.NBLK % self.PB == 0
        self.NG = self.NBLK // self.PB             # psum tile groups (2)


# ---------------------------------------------------------------- device IR

def build_program(cfg: Cfg):
    nc = bacc.Bacc("TRN2", target_bir_lowering=False, debug=False)
    fp = cfg.fp
    KD, NZT, CJP, NCJ, CP, NCH, B, J = (cfg.KD, cfg.NZT, cfg.CJP, cfg.NCJ,
                                        cfg.CP, cfg.NCH, cfg.B, cfg.J)
    NBW, PB, NG, L = cfg.NBLK_W, cfg.PB, cfg.NG, cfg.L
    D = cfg.D

    z = nc.dram_tensor("z", [NZT, 128, D], fp, kind="ExternalInput").ap()
    pt = nc.dram_tensor("pt", [D, CJP], fp, kind="ExternalInput").ap()
    rw = nc.dram_tensor("rw", [1, CJP], F32, kind="ExternalInput").ap()
    s01 = nc.dram_tensor("s01", [NCJ, 128, 128], F32, kind="ExternalInput").ap()
    bias2 = nc.dram_tensor("bias2", [NCH, 128], F32, kind="ExternalInput").ap()
    out = nc.dram_tensor("out", [NCH, 128, B], F32, kind="ExternalOutput").ap()
    wscratch = nc.dram_tensor("wscratch", [1, CJP], F32).ap()

    AF = mybir.ActivationFunctionType
    with tile.TileContext(nc) as tc:
        with tc.tile_pool(name="singles", bufs=1) as singles:
            zT = singles.tile([128, KD, cfg.BLP], fp)
            pt_sb = singles.tile([128, KD, CJP], fp)
            s_sb = singles.tile([128, NCJ, 128], F32)
            wv = singles.tile([128, NCJ], F32)
            bias_sb = singles.tile([128, NCH], F32)
            pooled = singles.tile([128, NCJ, B], F32)
            ones_sb = singles.tile([128, 1], fp)
            rw_sb = singles.tile([1, CJP], F32)
            e_row = singles.tile([1, CJP], F32)
            sp_row = singles.tile([1, CJP], F32)
            norm_row = singles.tile([1, CJP], F32)
            rp_row = singles.tile([1, CJP], F32)
            w_row = singles.tile([1, CJP], F32)
            out_sb = singles.tile([128, NCH, B], F32)

            # ---- constant loads (SWDGE so the HWDGE ring stays clear for
            # the xbar transposes)
            nc.scalar.dma_start(out=pt_sb,
                                in_=pt.rearrange("(k p) c -> p k c", p=128))
            nc.scalar.dma_start(out=s_sb, in_=s01.rearrange("t p c -> p t c"))
            nc.scalar.dma_start(out=rw_sb, in_=rw)
            nc.scalar.dma_start(out=bias_sb, in_=bias2.rearrange("h p -> p h"))
            nc.vector.memset(ones_sb, 1.0)

            # ---- prototype column sumsq -> w' = softplus(rw) / ||p||
            with (
                tc.tile_pool(name="p2", bufs=KD) as p2_pool,
                tc.tile_pool(name="ss_ps", bufs=1, space="PSUM") as ss_pool,
            ):
                ss_ps = ss_pool.tile([1, CJP], F32)
                p2s = []
                for k in range(KD):
                    p2 = p2_pool.tile([128, CJP], fp)
                    nc.scalar.activation(p2, pt_sb[:, k, :], AF.Square)
                    p2s.append(p2)
                for n0 in range(0, CJP, 512):
                    sl = slice(n0, min(n0 + 512, CJP))
                    for k in range(KD):
                        nc.tensor.matmul(ss_ps[0:1, sl], lhsT=ones_sb,
                                         rhs=p2s[k][:, sl],
                                         start=(k == 0), stop=(k == KD - 1))
                nc.scalar.activation(norm_row, ss_ps, AF.Sqrt)
            nc.vector.reciprocal(rp_row, norm_row)
            # softplus(x) = ln(exp(x) + 1)
            nc.scalar.activation(e_row, rw_sb, AF.Exp)
            nc.scalar.activation(sp_row, e_row, AF.Ln, bias=1.0)
            nc.vector.tensor_mul(w_row, sp_row, rp_row)
            # scatter [1, CJP] -> [128, NCJ] (cj = t*128 + p) via DRAM
            nc.scalar.dma_start(out=wscratch, in_=w_row)
            nc.scalar.dma_start(
                out=wv, in_=wscratch.rearrange("a (t p) -> (a p) t", p=128))
            for t in range(NCJ):
                nc.vector.tensor_scalar_mul(out=s_sb[:, t, :],
                                            in0=s_sb[:, t, :],
                                            scalar1=wv[:, t:t + 1])

            # ---- z: load, row-normalize, transpose into zT
            with (
                tc.tile_pool(name="zc", bufs=3) as zc_pool,
                tc.tile_pool(name="zn", bufs=3) as zn_pool,
                tc.tile_pool(name="sq", bufs=2) as sq_pool,
                tc.tile_pool(name="small", bufs=6) as small,
            ):
                for t in range(NZT):
                    zc = zc_pool.tile([128, D], fp)
                    nc.scalar.dma_start(out=zc, in_=z[t])
                    sq = sq_pool.tile([128, D], fp)
                    ss = small.tile([128, 1], F32, tag="ss")
                    nc.scalar.activation(sq, zc, AF.Square, accum_out=ss)
                    nz = small.tile([128, 1], F32, tag="nz")
                    nc.scalar.activation(nz, ss, AF.Sqrt)
                    rz = small.tile([128, 1], F32, tag="rz")
                    nc.vector.reciprocal(rz, nz)
                    zn = zn_pool.tile([128, D], fp)
                    nc.vector.tensor_scalar_mul(out=zn, in0=zc, scalar1=rz)
                    for k in range(KD):
                        nc.sync.dma_start_transpose(
                            zT[:, k, t * 128:(t + 1) * 128],
                            zn[:, k * 128:(k + 1) * 128])

            # ---- main: sim^T chunks + fused max-over-l
            with tc.tile_pool(name="mm", bufs=2, space="PSUM") as mm_pool:
                for ci in range(NCJ):
                    for g in range(NG):
                        ps = mm_pool.tile([128, PB * 512], F32)
                        for blk in range(PB):
                            col0 = (g * PB + blk) * NBW
                            for k in range(KD):
                                nc.tensor.matmul(
                                    ps[:, blk * 512: blk * 512 + NBW],
                                    lhsT=pt_sb[:, k, ci * 128:(ci + 1) * 128],
                                    rhs=zT[:, k, col0:col0 + NBW],
                                    start=(k == 0), stop=(k == KD - 1))
                        view = ps.rearrange("p (blk c) -> p blk c", blk=PB)
                        view = view[:, :, 0:NBW]
                        view = view.rearrange("p blk (s l) -> p blk s l", l=L)
                        nc.vector.reduce_max(
                            out=pooled[:, ci, g * PB * 2:(g + 1) * PB * 2],
                            in_=view, axis=mybir.AxisListType.X)

            # ---- j-sum via block-diagonal selector matmul + bias
            with tc.tile_pool(name="ops", bufs=NCH, space="PSUM") as out_pool:
                for h in range(NCH):
                    op = out_pool.tile([128, B], F32)
                    for i, t in enumerate(range(h * J, (h + 1) * J)):
                        nc.tensor.matmul(op, lhsT=s_sb[:, t, :],
                                         rhs=pooled[:, t, :],
                                         start=(i == 0), stop=(i == J - 1))
                    nc.scalar.activation(out_sb[:, h, :], op, AF.Identity,
                                         bias=bias_sb[:, h:h + 1])
            nc.sync.dma_start(out=out.rearrange("h p b -> p h b"), in_=out_sb)
    return nc


# ---------------------------------------------------------------- host side

def make_core_inputs(cfg: Cfg, spatial, prototypes, raw_weights, bias, ib, ic):
    """Build the input map for core (ib, ic) from FULL fp32 inputs."""
    npfp = np.float16 if cfg.fp == mybir.dt.float16 else np.float32
    B, L, D, C, J = cfg.B, cfg.L, cfg.D, cfg.C, cfg.J

    zs = spatial[ib * B:(ib + 1) * B].reshape(cfg.BL, D)
    z = np.ones((cfg.BLP, D), dtype=npfp)
    z[:cfg.BL] = zs.astype(npfp)
    z = z.reshape(cfg.NZT, 128, D)

    ps = prototypes[ic * C:(ic + 1) * C].reshape(C * J, D)
    pfull = np.ones((cfg.CJP, D), dtype=npfp)
    pfull[:C * J] = ps.astype(npfp)
    pt = np.ascontiguousarray(pfull.T)                      # [D, CJP]

    rw = np.zeros((1, cfg.CJP), dtype=np.float32)
    rw[0, :C * J] = raw_weights[ic * C:(ic + 1) * C].reshape(-1)

    b2 = np.zeros((cfg.NCH, 128), dtype=np.float32)
    b2.reshape(-1)[:C] = bias[ic * C:(ic + 1) * C]

    return {"z": z, "pt": pt, "rw": rw, "b": b2}


def make_s01(cfg: Cfg):
    """[NCJ, 128, 128] 0/1 selector: chunk t row p -> local class column."""
    s = np.zeros((cfg.NCJ, 128, 128), dtype=np.float32)
    for t in range(cfg.NCJ):
        h = t // cfg.J
        cj = t * 128 + np.arange(128)
        c_local = cj // cfg.J - h * 128
        s[t, np.arange(128), c_local] = 1.0
    return s


_CACHE = {}


def _get_program(key="main"):
    if key not in _CACHE:
        cfg = Cfg()
        nc = build_program(cfg)
        nc.finalize()          # run bacc lowering (wait-splitting, reg alloc)
        _CACHE[key] = (cfg, nc)
    return _CACHE[key]


def _run(spatial, prototypes, raw_weights, bias, **spmd_kwargs):
    spatial = np.asarray(spatial, dtype=np.float32)
    prototypes = np.asarray(prototypes, dtype=np.float32)
    raw_weights = np.asarray(raw_weights, dtype=np.float32)
    bias = np.asarray(bias, dtype=np.float32)

    cfg, nc = _get_program()
    s01 = make_s01(cfg)
    in_maps = []
    for core in range(cfg.NB * cfg.NC):
        ib, ic = core // cfg.NC, core % cfg.NC
        m = make_core_inputs(cfg, spatial, prototypes, raw_weights, bias,
                             ib, ic)
        in_maps.append({"z": m["z"], "pt": m["pt"], "rw": m["rw"],
                        "s01": s01, "bias2": m["b"]})

    res = run_bass_kernel_spmd(nc, in_maps,
                               core_ids=list(range(cfg.NB * cfg.NC)),
                               **spmd_kwargs)

    b_full = cfg.NB * cfg.B
    c_full = cfg.NC * cfg.C
    outp = np.zeros((b_full, c_full), dtype=np.float32)
    for core in range(cfg.NB * cfg.NC):
        ib, ic = core // cfg.NC, core % cfg.NC
        o = res.results[core]["out"].reshape(cfg.CP, cfg.B)    # [c_pad, b]
        outp[ib * cfg.B:(ib + 1) * cfg.B,
             ic * cfg.C:(ic + 1) * cfg.C] = o[:cfg.C].T
    return outp, res


def kernel(spatial, prototypes, raw_weights, bias):
    outp, _ = _run(spatial, prototypes, raw_weights, bias)
    return outp


# revision 14
# speedup vs baseline: 1.3424x; 1.3424x over previous
"""Trainium2 Bass kernel for nn_PrototypicalHead.

Reference computation (per full problem):
    z = l2norm(spatial, axis=-1)            # [b, l, d]
    p = l2norm(prototypes, axis=-1)         # [c, j, d]
    sim = einsum('bld,cjd->blcj', z, p)
    pooled = max over l                     # [b, c, j]
    out = sum_j pooled * softplus(raw_w) + bias   # [b, c]

Full shapes: b=32, l=196, d=768, c=1000, j=10.

Sharding: 2-way data parallel over b x 4-way tensor parallel over c
(8 cores).  Per core: b_sh=16 (bl=3136 rows), c_sh=250 (cj=2500 cols).

Per-core device pipeline:
  - load z natural [bl, d] fp16; row sumsq (ACT Square+accum); sqrt;
    reciprocal; scale rows (DVE tensor_scalar); batched DMA-xbar
    transpose ([128,768] -> [128,6,128] blockwise) into two half
    buffers zT_h [d, bl-half] fp16 so the main loop can start after
    the first half is transposed.
  - pT [d, cj] fed pre-transposed from host (fp16, raw).  Prototype
    normalization is folded into the per-(c,j) weights:
        w' = softplus(raw_w) / ||p||
    (max over l commutes with a positive per-(c,j) scale).  Column
    sumsq of pT via DVE square + ones-matmul.
  - main matmul: for each bl half, for each cj chunk of 128,
    psum[cj=128, 4 banks x 392] accumulated over 6 k-chunks (fp16).
    392 = 2*196: each PSUM bank holds exactly two l-segments.
  - max over l: one DVE reduce_max per 4-bank psum tile with a 4D AP
    [128, 4, 2, 196] -> [128, 8].
  - j-sum: block-diagonal selector matmul (S = w' scattered onto the
    (cj -> class) incidence pattern); 128 classes * 10 j = 10 cj-chunks
    align exactly with class-halves of 128.
  - bias add on ACT during PSUM->SBUF copy; single DMA out.

kernel() accepts FULL inputs and returns the FULL [32, 1000] fp32 output.
"""

import math

import numpy as np

import concourse.bass as bass
from concourse import bacc
import concourse.mybir as mybir
import concourse.tile as tile
from concourse.bass_utils import run_bass_kernel_spmd

F32 = mybir.dt.float32

# ---------------------------------------------------------------- config

class Cfg:
    def __init__(self, B=16, L=196, D=768, C=250, J=10, NB=2, NC=4,
                 fp=mybir.dt.float16, pb=4):
        self.B, self.L, self.D, self.C, self.J = B, L, D, C, J
        self.NB, self.NC = NB, NC          # mesh: batch shards x class shards
        self.fp = fp
        self.BL = B * L                    # rows of z per core (3136)
        self.KD = D // 128                 # k chunks (6)
        assert D % 128 == 0
        self.NZT = math.ceil(self.BL / 128)        # z row chunks (25)
        self.BLP = self.NZT * 128                  # padded rows (3200)
        self.NCJ = math.ceil(C * J / 128)          # cj chunks (20)
        self.CJP = self.NCJ * 128                  # padded cj (2560)
        assert self.CJP % J == 0
        self.CP = self.CJP // J                    # padded classes (256)
        assert self.CP % 128 == 0
        self.NCH = self.CP // 128                  # class halves (2)
        assert self.NCJ == self.NCH * J
        self.NBLK_W = 2 * L                        # psum block width (392)
        assert self.NBLK_W <= 512
        assert self.BL % self.NBLK_W == 0
        self.NBLK = self.BL // self.NBLK_W         # bl blocks (8)
        self.PB = min(pb, self.NBLK)               # blocks per psum tile (4)
        assert self.NBLK % self.PB == 0
        self.NG = self.NBLK // self.PB             # bl groups / zT halves (2)
        # zT half h holds z-chunks [t_lo[h], t_hi[h]] (inclusive); a chunk
        # straddling a group boundary is transposed into both halves.
        self.t_lo, self.t_hi = [], []
        w = self.PB * self.NBLK_W                  # cols per group (1568)
        for h in range(self.NG):
            self.t_lo.append((h * w) // 128)
            self.t_hi.append(min(self.NZT - 1,
                                 math.ceil((h + 1) * w / 128) - 1))


# ---------------------------------------------------------------- device IR

def build_program(cfg: Cfg):
    nc = bacc.Bacc("TRN2", target_bir_lowering=False, debug=False)
    fp = cfg.fp
    KD, NZT, CJP, NCJ, CP, NCH, B, J = (cfg.KD, cfg.NZT, cfg.CJP, cfg.NCJ,
                                        cfg.CP, cfg.NCH, cfg.B, cfg.J)
    NBW, PB, NG, L = cfg.NBLK_W, cfg.PB, cfg.NG, cfg.L
    D = cfg.D

    z = nc.dram_tensor("z", [NZT, 128, D], fp, kind="ExternalInput").ap()
    pt = nc.dram_tensor("pt", [D, CJP], fp, kind="ExternalInput").ap()
    rw = nc.dram_tensor("rw", [1, CJP], F32, kind="ExternalInput").ap()
    s01 = nc.dram_tensor("s01", [NCJ, 128, 128], F32, kind="ExternalInput").ap()
    bias2 = nc.dram_tensor("bias2", [NCH, 128], F32, kind="ExternalInput").ap()
    out = nc.dram_tensor("out", [NCH, 128, B], F32, kind="ExternalOutput").ap()
    wscratch = nc.dram_tensor("wscratch", [1, CJP], F32).ap()

    AF = mybir.ActivationFunctionType
    with tile.TileContext(nc) as tc:
        with tc.tile_pool(name="singles", bufs=1) as singles:
            zTs = [singles.tile([128, KD, (cfg.t_hi[h] - cfg.t_lo[h] + 1) * 128],
                                fp, name=f"zT{h}", tag=f"zT{h}")
                   for h in range(NG)]
            pt_sb = singles.tile([128, KD, CJP], fp)
            s_sb = singles.tile([128, NCJ, 128], F32)
            wv = singles.tile([128, NCJ], F32)
            bias_sb = singles.tile([128, NCH], F32)
            pooled = singles.tile([128, NCJ, B], F32)
            ones_sb = singles.tile([128, 1], fp)
            rw_sb = singles.tile([1, CJP], F32)
            e_row = singles.tile([1, CJP], F32)
            sp_row = singles.tile([1, CJP], F32)
            norm_row = singles.tile([1, CJP], F32)
            rp_row = singles.tile([1, CJP], F32)
            w_row = singles.tile([1, CJP], F32)
            out_sb = singles.tile([128, NCH, B], F32)

            # ---- input loads, all on SWDGE (gpsimd) so the sync HWDGE
            # ring carries only the xbar transposes.
            nc.gpsimd.dma_start(out=pt_sb,
                                in_=pt.rearrange("(k p) c -> p k c", p=128))
            nc.gpsimd.dma_start(out=s_sb, in_=s01.rearrange("t p c -> p t c"))
            nc.gpsimd.dma_start(out=rw_sb, in_=rw)
            nc.gpsimd.dma_start(out=bias_sb, in_=bias2.rearrange("h p -> p h"))
            nc.vector.memset(ones_sb, 1.0)

            # ---- z: load, row-normalize, batched transpose into halves
            with (
                tc.tile_pool(name="zc", bufs=4) as zc_pool,
                tc.tile_pool(name="zn", bufs=4) as zn_pool,
                tc.tile_pool(name="sq", bufs=3) as sq_pool,
                tc.tile_pool(name="small", bufs=8) as small,
            ):
                for t in range(NZT):
                    zc = zc_pool.tile([128, D], fp)
                    nc.gpsimd.dma_start(out=zc, in_=z[t])
                    sq = sq_pool.tile([128, D], fp)
                    ss = small.tile([128, 1], F32, tag="ss")
                    nc.scalar.activation(sq, zc, AF.Square, accum_out=ss)
                    nz = small.tile([128, 1], F32, tag="nz")
                    nc.scalar.activation(nz, ss, AF.Sqrt)
                    rz = small.tile([128, 1], F32, tag="rz")
                    nc.vector.reciprocal(rz, nz)
                    zn = zn_pool.tile([128, D], fp)
                    nc.vector.tensor_scalar_mul(out=zn, in0=zc, scalar1=rz)
                    for h in range(NG):
                        if cfg.t_lo[h] <= t <= cfg.t_hi[h]:
                            lt = t - cfg.t_lo[h]
                            nc.sync.dma_start_transpose(
                                zTs[h][:, :, lt * 128:(lt + 1) * 128], zn)

            # ---- prototype column sumsq -> w' = softplus(rw) / ||p||
            with (
                tc.tile_pool(name="p2", bufs=KD) as p2_pool,
                tc.tile_pool(name="ss_ps", bufs=1, space="PSUM") as ss_pool,
            ):
                ss_ps = ss_pool.tile([1, CJP], F32)
                p2s = []
                for k in range(KD):
                    p2 = p2_pool.tile([128, CJP], fp)
                    nc.vector.tensor_mul(p2, pt_sb[:, k, :], pt_sb[:, k, :])
                    p2s.append(p2)
                for n0 in range(0, CJP, 512):
                    sl = slice(n0, min(n0 + 512, CJP))
                    for k in range(KD):
                        nc.tensor.matmul(ss_ps[0:1, sl], lhsT=ones_sb,
                                         rhs=p2s[k][:, sl],
                                         start=(k == 0), stop=(k == KD - 1))
                nc.scalar.activation(norm_row, ss_ps, AF.Sqrt)
            nc.vector.reciprocal(rp_row, norm_row)
            # softplus(x) = ln(exp(x) + 1)
            nc.scalar.activation(e_row, rw_sb, AF.Exp)
            nc.scalar.activation(sp_row, e_row, AF.Ln, bias=1.0)
            nc.vector.tensor_mul(w_row, sp_row, rp_row)
            # scatter [1, CJP] -> [128, NCJ] (cj = t*128 + p) via DRAM
            nc.gpsimd.dma_start(out=wscratch, in_=w_row)
            nc.gpsimd.dma_start(
                out=wv, in_=wscratch.rearrange("a (t p) -> (a p) t", p=128))
            for t in range(NCJ):
                nc.vector.tensor_scalar_mul(out=s_sb[:, t, :],
                                            in0=s_sb[:, t, :],
                                            scalar1=wv[:, t:t + 1])

            # ---- main: sim^T chunks + fused max-over-l, one bl-half at
            # a time (half h only needs zTs[h]); k outer within a psum
            # tile so the 4 consecutive matmuls share the stationary.
            with tc.tile_pool(name="mm", bufs=2, space="PSUM") as mm_pool:
                for h in range(NG):
                    for ci in range(NCJ):
                        ps = mm_pool.tile([128, PB * 512], F32)
                        for k in range(KD):
                            for blk in range(PB):
                                col0 = ((h * PB + blk) * NBW
                                        - cfg.t_lo[h] * 128)
                                nc.tensor.matmul(
                                    ps[:, blk * 512: blk * 512 + NBW],
                                    lhsT=pt_sb[:, k, ci * 128:(ci + 1) * 128],
                                    rhs=zTs[h][:, k, col0:col0 + NBW],
                                    start=(k == 0), stop=(k == KD - 1),
                                    skip_group_check=True)
                        view = ps.rearrange("p (blk c) -> p blk c", blk=PB)
                        view = view[:, :, 0:NBW]
                        view = view.rearrange("p blk (s l) -> p blk s l", l=L)
                        nc.vector.reduce_max(
                            out=pooled[:, ci, h * PB * 2:(h + 1) * PB * 2],
                            in_=view, axis=mybir.AxisListType.X)

            # ---- j-sum via block-diagonal selector matmul + bias
            with tc.tile_pool(name="ops", bufs=NCH, space="PSUM") as out_pool:
                for h in range(NCH):
                    op = out_pool.tile([128, B], F32)
                    for i, t in enumerate(range(h * J, (h + 1) * J)):
                        nc.tensor.matmul(op, lhsT=s_sb[:, t, :],
                                         rhs=pooled[:, t, :],
                                         start=(i == 0), stop=(i == J - 1))
                    nc.scalar.activation(out_sb[:, h, :], op, AF.Identity,
                                         bias=bias_sb[:, h:h + 1])
            nc.sync.dma_start(out=out.rearrange("h p b -> p h b"), in_=out_sb)
    return nc


# ---------------------------------------------------------------- host side

def make_core_inputs(cfg: Cfg, spatial, prototypes, raw_weights, bias, ib, ic):
    """Build the input map for core (ib, ic) from FULL fp32 inputs."""
    npfp = np.float16 if cfg.fp == mybir.dt.float16 else np.float32
    B, L, D, C, J = cfg.B, cfg.L, cfg.D, cfg.C, cfg.J

    zs = spatial[ib * B:(ib + 1) * B].reshape(cfg.BL, D)
    z = np.ones((cfg.BLP, D), dtype=npfp)
    z[:cfg.BL] = zs.astype(npfp)
    z = z.reshape(cfg.NZT, 128, D)

    ps = prototypes[ic * C:(ic + 1) * C].reshape(C * J, D)
    pfull = np.ones((cfg.CJP, D), dtype=npfp)
    pfull[:C * J] = ps.astype(npfp)
    pt = np.ascontiguousarray(pfull.T)                      # [D, CJP]

    rw = np.zeros((1, cfg.CJP), dtype=np.float32)
    rw[0, :C * J] = raw_weights[ic * C:(ic + 1) * C].reshape(-1)

    b2 = np.zeros((cfg.NCH, 128), dtype=np.float32)
    b2.reshape(-1)[:C] = bias[ic * C:(ic + 1) * C]

    return {"z": z, "pt": pt, "rw": rw, "b": b2}


def make_s01(cfg: Cfg):
    """[NCJ, 128, 128] 0/1 selector: chunk t row p -> local class column."""
    s = np.zeros((cfg.NCJ, 128, 128), dtype=np.float32)
    for t in range(cfg.NCJ):
        h = t // cfg.J
        cj = t * 128 + np.arange(128)
        c_local = cj // cfg.J - h * 128
        s[t, np.arange(128), c_local] = 1.0
    return s


_CACHE = {}


def _get_program(key="main"):
    if key not in _CACHE:
        cfg = Cfg()
        nc = build_program(cfg)
        nc.finalize()          # run bacc lowering (wait-splitting, reg alloc)
        _CACHE[key] = (cfg, nc)
    return _CACHE[key]


def _run(spatial, prototypes, raw_weights, bias, **spmd_kwargs):
    spatial = np.asarray(spatial, dtype=np.float32)
    prototypes = np.asarray(prototypes, dtype=np.float32)
    raw_weights = np.asarray(raw_weights, dtype=np.float32)
    bias = np.asarray(bias, dtype=np.float32)

    cfg, nc = _get_program()
    s01 = make_s01(cfg)
    in_maps = []
    for core in range(cfg.NB * cfg.NC):
        ib, ic = core // cfg.NC, core % cfg.NC
        m = make_core_inputs(cfg, spatial, prototypes, raw_weights, bias,
                             ib, ic)
        in_maps.append({"z": m["z"], "pt": m["pt"], "rw": m["rw"],
                        "s01": s01, "bias2": m["b"]})

    res = run_bass_kernel_spmd(nc, in_maps,
                               core_ids=list(range(cfg.NB * cfg.NC)),
                               **spmd_kwargs)

    b_full = cfg.NB * cfg.B
    c_full = cfg.NC * cfg.C
    outp = np.zeros((b_full, c_full), dtype=np.float32)
    for core in range(cfg.NB * cfg.NC):
        ib, ic = core // cfg.NC, core % cfg.NC
        o = res.results[core]["out"].reshape(cfg.CP, cfg.B)    # [c_pad, b]
        outp[ib * cfg.B:(ib + 1) * cfg.B,
             ic * cfg.C:(ic + 1) * cfg.C] = o[:cfg.C].T
    return outp, res


def kernel(spatial, prototypes, raw_weights, bias):
    outp, _ = _run(spatial, prototypes, raw_weights, bias)
    return outp


# revision 16
# speedup vs baseline: 1.8530x; 1.3803x over previous
"""Trainium2 Bass kernel for nn_PrototypicalHead.

Reference computation (per full problem):
    z = l2norm(spatial, axis=-1)            # [b, l, d]
    p = l2norm(prototypes, axis=-1)         # [c, j, d]
    sim = einsum('bld,cjd->blcj', z, p)
    pooled = max over l                     # [b, c, j]
    out = sum_j pooled * softplus(raw_w) + bias   # [b, c]

Full shapes: b=32, l=196, d=768, c=1000, j=10.

Sharding: 2-way data parallel over b x 4-way tensor parallel over c
(8 cores).  Per core: b_sh=16 (bl=3136 rows), c_sh=250 (cj=2500 cols).

Per-core device pipeline:
  - load z natural [bl, d] fp16; row sumsq (ACT Square+accum); sqrt;
    reciprocal; scale rows (DVE tensor_scalar); batched DMA-xbar
    transpose ([128,768] -> [128,6,128] blockwise) into two half
    buffers zT_h [d, bl-half] fp16 so the main loop can start after
    the first half is transposed.
  - pT [d, cj] fed pre-transposed from host (fp16, raw).  Prototype
    normalization is folded into the per-(c,j) weights:
        w' = softplus(raw_w) / ||p||
    (max over l commutes with a positive per-(c,j) scale).  Column
    sumsq of pT via DVE square + ones-matmul.
  - main matmul: for each bl half, for each cj chunk of 128,
    psum[cj=128, 4 banks x 392] accumulated over 6 k-chunks (fp16).
    392 = 2*196: each PSUM bank holds exactly two l-segments.
  - max over l: one DVE reduce_max per 4-bank psum tile with a 4D AP
    [128, 4, 2, 196] -> [128, 8].
  - j-sum: block-diagonal selector matmul (S = w' scattered onto the
    (cj -> class) incidence pattern); 128 classes * 10 j = 10 cj-chunks
    align exactly with class-halves of 128.
  - bias add on ACT during PSUM->SBUF copy; single DMA out.

kernel() accepts FULL inputs and returns the FULL [32, 1000] fp32 output.
"""

import math

import numpy as np

import concourse.bass as bass
from concourse import bacc
import concourse.mybir as mybir
import concourse.tile as tile
from concourse.bass_utils import run_bass_kernel_spmd

F32 = mybir.dt.float32

# ---------------------------------------------------------------- config

class Cfg:
    def __init__(self, B=16, L=196, D=768, C=250, J=10, NB=2, NC=4,
                 fp=mybir.dt.float16, pb=4):
        self.B, self.L, self.D, self.C, self.J = B, L, D, C, J
        self.NB, self.NC = NB, NC          # mesh: batch shards x class shards
        self.fp = fp
        self.BL = B * L                    # rows of z per core (3136)
        self.KD = D // 128                 # k chunks (6)
        assert D % 128 == 0
        self.NZT = math.ceil(self.BL / 128)        # z row chunks (25)
        self.BLP = self.NZT * 128                  # padded rows (3200)
        self.NCJ = math.ceil(C * J / 128)          # cj chunks (20)
        self.CJP = self.NCJ * 128                  # padded cj (2560)
        assert self.CJP % J == 0
        self.CP = self.CJP // J                    # padded classes (256)
        assert self.CP % 128 == 0
        self.NCH = self.CP // 128                  # class halves (2)
        assert self.NCJ == self.NCH * J
        self.NBLK_W = 2 * L                        # psum block width (392)
        assert self.NBLK_W <= 512
        assert self.BL % self.NBLK_W == 0
        self.NBLK = self.BL // self.NBLK_W         # bl blocks (8)
        self.PB = min(pb, self.NBLK)               # blocks per psum tile (4)
        assert self.NBLK % self.PB == 0
        self.NG = self.NBLK // self.PB             # bl groups / zT halves (2)
        # zT half h holds z-chunks [t_lo[h], t_hi[h]] (inclusive); a chunk
        # straddling a group boundary is transposed into both halves.
        self.t_lo, self.t_hi = [], []
        w = self.PB * self.NBLK_W                  # cols per group (1568)
        for h in range(self.NG):
            self.t_lo.append((h * w) // 128)
            self.t_hi.append(min(self.NZT - 1,
                                 math.ceil((h + 1) * w / 128) - 1))


# ---------------------------------------------------------------- device IR

def build_program(cfg: Cfg):
    nc = bacc.Bacc("TRN2", target_bir_lowering=False, debug=False)
    fp = cfg.fp
    KD, NZT, CJP, NCJ, CP, NCH, B, J = (cfg.KD, cfg.NZT, cfg.CJP, cfg.NCJ,
                                        cfg.CP, cfg.NCH, cfg.B, cfg.J)
    NBW, PB, NG, L = cfg.NBLK_W, cfg.PB, cfg.NG, cfg.L
    D = cfg.D

    z = nc.dram_tensor("z", [NZT, 128, D], fp, kind="ExternalInput").ap()
    pt = nc.dram_tensor("pt", [D, CJP], fp, kind="ExternalInput").ap()
    rw = nc.dram_tensor("rw", [1, CJP], F32, kind="ExternalInput").ap()
    s01 = nc.dram_tensor("s01", [NCJ, 128, 128], F32, kind="ExternalInput").ap()
    bias2 = nc.dram_tensor("bias2", [NCH, 128], F32, kind="ExternalInput").ap()
    out = nc.dram_tensor("out", [NCH, 128, B], F32, kind="ExternalOutput").ap()
    wscratch = nc.dram_tensor("wscratch", [1, CJP], F32).ap()

    AF = mybir.ActivationFunctionType
    with tile.TileContext(nc) as tc:
        with tc.tile_pool(name="singles", bufs=1) as singles:
            zTs = [singles.tile([128, KD, (cfg.t_hi[h] - cfg.t_lo[h] + 1) * 128],
                                fp, name=f"zT{h}", tag=f"zT{h}")
                   for h in range(NG)]
            pt_sb = singles.tile([128, KD, CJP], fp)
            s_sb = singles.tile([128, NCJ, 128], F32)
            wv = singles.tile([128, NCJ], F32)
            bias_sb = singles.tile([128, NCH], F32)
            pooled = singles.tile([128, NCJ, B], F32)
            ones_sb = singles.tile([128, 1], fp)
            rw_sb = singles.tile([1, CJP], F32)
            e_row = singles.tile([1, CJP], F32)
            sp_row = singles.tile([1, CJP], F32)
            norm_row = singles.tile([1, CJP], F32)
            rp_row = singles.tile([1, CJP], F32)
            w_row = singles.tile([1, CJP], F32)
            out_sb = singles.tile([128, NCH, B], F32)

            # ---- input loads, all on SWDGE (gpsimd) so the sync HWDGE
            # ring carries only the xbar transposes.  pt is loaded per
            # k-chunk so the p^2 pipeline starts before the full load.
            for k in range(KD):
                nc.gpsimd.dma_start(
                    out=pt_sb[:, k, :],
                    in_=pt.rearrange("(k p) c -> p k c", p=128)[:, k, :])
            nc.gpsimd.dma_start(out=rw_sb, in_=rw)
            nc.gpsimd.dma_start(out=bias_sb, in_=bias2.rearrange("h p -> p h"))
            nc.gpsimd.dma_start(out=s_sb, in_=s01.rearrange("t p c -> p t c"))
            nc.vector.memset(ones_sb, 1.0)

            # ---- z: load all chunks, batched row-normalize, batched
            # xbar transpose into the two half buffers.
            zcs = []
            with (
                tc.tile_pool(name="zc", bufs=1) as zc_pool,
                tc.tile_pool(name="zn", bufs=6) as zn_pool,
                tc.tile_pool(name="sq", bufs=3) as sq_pool,
            ):
                ss_all = singles.tile([128, NZT], F32)
                nz_all = singles.tile([128, NZT], F32)
                rz_all = singles.tile([128, NZT], F32)
                for t in range(NZT):
                    zc = zc_pool.tile([128, D], fp, tag=f"zc{t}")
                    nc.gpsimd.dma_start(out=zc, in_=z[t])
                    zcs.append(zc)
                for t in range(NZT):
                    sq = sq_pool.tile([128, D], fp)
                    nc.scalar.activation(sq, zcs[t], AF.Square,
                                         accum_out=ss_all[:, t:t + 1])
                    # sqrt+recip batched over groups of chunks to keep
                    # the transpose pipeline fed with low op overhead
                    if t == NZT - 1 or t == cfg.t_hi[0]:
                        lo = 0 if t == cfg.t_hi[0] else cfg.t_hi[0] + 1
                        sl = slice(lo, t + 1)
                        nc.scalar.activation(nz_all[:, sl], ss_all[:, sl],
                                             AF.Sqrt)
                        nc.vector.reciprocal(rz_all[:, sl], nz_all[:, sl])
                for t in range(NZT):
                    zn = zn_pool.tile([128, D], fp)
                    nc.vector.tensor_scalar_mul(out=zn, in0=zcs[t],
                                                scalar1=rz_all[:, t:t + 1])
                    for h in range(NG):
                        if cfg.t_lo[h] <= t <= cfg.t_hi[h]:
                            lt = t - cfg.t_lo[h]
                            nc.sync.dma_start_transpose(
                                zTs[h][:, :, lt * 128:(lt + 1) * 128], zn)

            # ---- prototype column sumsq -> w' = softplus(rw) / ||p||
            with (
                tc.tile_pool(name="p2", bufs=KD) as p2_pool,
                tc.tile_pool(name="ss_ps", bufs=1, space="PSUM") as ss_pool,
            ):
                ss_ps = ss_pool.tile([1, CJP], F32)
                p2s = []
                for k in range(KD):
                    p2 = p2_pool.tile([128, CJP], fp)
                    nc.vector.tensor_mul(p2, pt_sb[:, k, :], pt_sb[:, k, :])
                    p2s.append(p2)
                for n0 in range(0, CJP, 512):
                    sl = slice(n0, min(n0 + 512, CJP))
                    for k in range(KD):
                        nc.tensor.matmul(ss_ps[0:1, sl], lhsT=ones_sb,
                                         rhs=p2s[k][:, sl],
                                         start=(k == 0), stop=(k == KD - 1))
                nc.scalar.activation(norm_row, ss_ps, AF.Sqrt)
            nc.vector.reciprocal(rp_row, norm_row)
            # softplus(x) = ln(exp(x) + 1)
            nc.scalar.activation(e_row, rw_sb, AF.Exp)
            nc.scalar.activation(sp_row, e_row, AF.Ln, bias=1.0)
            nc.vector.tensor_mul(w_row, sp_row, rp_row)
            # scatter [1, CJP] -> [128, NCJ] (cj = t*128 + p) via DRAM
            nc.gpsimd.dma_start(out=wscratch, in_=w_row)
            nc.gpsimd.dma_start(
                out=wv, in_=wscratch.rearrange("a (t p) -> (a p) t", p=128))
            for t in range(NCJ):
                nc.vector.tensor_scalar_mul(out=s_sb[:, t, :],
                                            in0=s_sb[:, t, :],
                                            scalar1=wv[:, t:t + 1])

            # ---- main: sim^T chunks + fused max-over-l, one bl-half at
            # a time (half h only needs zTs[h]); k outer within a psum
            # tile so the 4 consecutive matmuls share the stationary.
            with tc.tile_pool(name="mm", bufs=2, space="PSUM") as mm_pool:
                for h in range(NG):
                    for ci in range(NCJ):
                        ps = mm_pool.tile([128, PB * 512], F32)
                        for k in range(KD):
                            for blk in range(PB):
                                col0 = ((h * PB + blk) * NBW
                                        - cfg.t_lo[h] * 128)
                                nc.tensor.matmul(
                                    ps[:, blk * 512: blk * 512 + NBW],
                                    lhsT=pt_sb[:, k, ci * 128:(ci + 1) * 128],
                                    rhs=zTs[h][:, k, col0:col0 + NBW],
                                    start=(k == 0), stop=(k == KD - 1),
                                    skip_group_check=True)
                        view = ps.rearrange("p (blk c) -> p blk c", blk=PB)
                        view = view[:, :, 0:NBW]
                        view = view.rearrange("p blk (s l) -> p blk s l", l=L)
                        nc.vector.reduce_max(
                            out=pooled[:, ci, h * PB * 2:(h + 1) * PB * 2],
                            in_=view, axis=mybir.AxisListType.X)

            # ---- j-sum via block-diagonal selector matmul + bias
            with tc.tile_pool(name="ops", bufs=NCH, space="PSUM") as out_pool:
                for h in range(NCH):
                    op = out_pool.tile([128, B], F32)
                    for i, t in enumerate(range(h * J, (h + 1) * J)):
                        nc.tensor.matmul(op, lhsT=s_sb[:, t, :],
                                         rhs=pooled[:, t, :],
                                         start=(i == 0), stop=(i == J - 1))
                    nc.scalar.activation(out_sb[:, h, :], op, AF.Identity,
                                         bias=bias_sb[:, h:h + 1])
            nc.sync.dma_start(out=out.rearrange("h p b -> p h b"), in_=out_sb)
    return nc


# ---------------------------------------------------------------- host side

def make_core_inputs(cfg: Cfg, spatial, prototypes, raw_weights, bias, ib, ic):
    """Build the input map for core (ib, ic) from FULL fp32 inputs."""
    npfp = np.float16 if cfg.fp == mybir.dt.float16 else np.float32
    B, L, D, C, J = cfg.B, cfg.L, cfg.D, cfg.C, cfg.J

    zs = spatial[ib * B:(ib + 1) * B].reshape(cfg.BL, D)
    z = np.ones((cfg.BLP, D), dtype=npfp)
    z[:cfg.BL] = zs.astype(npfp)
    z = z.reshape(cfg.NZT, 128, D)

    ps = prototypes[ic * C:(ic + 1) * C].reshape(C * J, D)
    pfull = np.ones((cfg.CJP, D), dtype=npfp)
    pfull[:C * J] = ps.astype(npfp)
    pt = np.ascontiguousarray(pfull.T)                      # [D, CJP]

    rw = np.zeros((1, cfg.CJP), dtype=np.float32)
    rw[0, :C * J] = raw_weights[ic * C:(ic + 1) * C].reshape(-1)

    b2 = np.zeros((cfg.NCH, 128), dtype=np.float32)
    b2.reshape(-1)[:C] = bias[ic * C:(ic + 1) * C]

    return {"z": z, "pt": pt, "rw": rw, "b": b2}


def make_s01(cfg: Cfg):
    """[NCJ, 128, 128] 0/1 selector: chunk t row p -> local class column."""
    s = np.zeros((cfg.NCJ, 128, 128), dtype=np.float32)
    for t in range(cfg.NCJ):
        h = t // cfg.J
        cj = t * 128 + np.arange(128)
        c_local = cj // cfg.J - h * 128
        s[t, np.arange(128), c_local] = 1.0
    return s


_CACHE = {}


def _get_program(key="main"):
    if key not in _CACHE:
        cfg = Cfg()
        nc = build_program(cfg)
        nc.finalize()          # run bacc lowering (wait-splitting, reg alloc)
        _CACHE[key] = (cfg, nc)
    return _CACHE[key]


def _run(spatial, prototypes, raw_weights, bias, **spmd_kwargs):
    spatial = np.asarray(spatial, dtype=np.float32)
    prototypes = np.asarray(prototypes, dtype=np.float32)
    raw_weights = np.asarray(raw_weights, dtype=np.float32)
    bias = np.asarray(bias, dtype=np.float32)

    cfg, nc = _get_program()
    s01 = make_s01(cfg)
    in_maps = []
    for core in range(cfg.NB * cfg.NC):
        ib, ic = core // cfg.NC, core % cfg.NC
        m = make_core_inputs(cfg, spatial, prototypes, raw_weights, bias,
                             ib, ic)
        in_maps.append({"z": m["z"], "pt": m["pt"], "rw": m["rw"],
                        "s01": s01, "bias2": m["b"]})

    res = run_bass_kernel_spmd(nc, in_maps,
                               core_ids=list(range(cfg.NB * cfg.NC)),
                               **spmd_kwargs)

    b_full = cfg.NB * cfg.B
    c_full = cfg.NC * cfg.C
    outp = np.zeros((b_full, c_full), dtype=np.float32)
    for core in range(cfg.NB * cfg.NC):
        ib, ic = core // cfg.NC, core % cfg.NC
        o = res.results[core]["out"].reshape(cfg.CP, cfg.B)    # [c_pad, b]
        outp[ib * cfg.B:(ib + 1) * cfg.B,
             ic * cfg.C:(ic + 1) * cfg.C] = o[:cfg.C].T
    return outp, res


def kernel(spatial, prototypes, raw_weights, bias):
    outp, _ = _run(spatial, prototypes, raw_weights, bias)
    return outp


# revision 17
# speedup vs baseline: 1.9420x; 1.0480x over previous
"""Trainium2 Bass kernel for nn_PrototypicalHead.

Reference computation (per full problem):
    z = l2norm(spatial, axis=-1)            # [b, l, d]
    p = l2norm(prototypes, axis=-1)         # [c, j, d]
    sim = einsum('bld,cjd->blcj', z, p)
    pooled = max over l                     # [b, c, j]
    out = sum_j pooled * softplus(raw_w) + bias   # [b, c]

Full shapes: b=32, l=196, d=768, c=1000, j=10.

Sharding: 2-way data parallel over b x 4-way tensor parallel over c
(8 cores).  Per core: b_sh=16 (bl=3136 rows), c_sh=250 (cj=2500 cols).

Per-core device pipeline:
  - load z natural [bl, d] fp16; row sumsq (ACT Square+accum); sqrt;
    reciprocal; scale rows (DVE tensor_scalar); batched DMA-xbar
    transpose ([128,768] -> [128,6,128] blockwise) into two half
    buffers zT_h [d, bl-half] fp16 so the main loop can start after
    the first half is transposed.
  - pT [d, cj] fed pre-transposed from host (fp16, raw).  Prototype
    normalization is folded into the per-(c,j) weights:
        w' = softplus(raw_w) / ||p||
    (max over l commutes with a positive per-(c,j) scale).  Column
    sumsq of pT via DVE square + ones-matmul.
  - main matmul: for each bl half, for each cj chunk of 128,
    psum[cj=128, 4 banks x 392] accumulated over 6 k-chunks (fp16).
    392 = 2*196: each PSUM bank holds exactly two l-segments.
  - max over l: one DVE reduce_max per 4-bank psum tile with a 4D AP
    [128, 4, 2, 196] -> [128, 8].
  - j-sum: block-diagonal selector matmul (S = w' scattered onto the
    (cj -> class) incidence pattern); 128 classes * 10 j = 10 cj-chunks
    align exactly with class-halves of 128.
  - bias add on ACT during PSUM->SBUF copy; single DMA out.

kernel() accepts FULL inputs and returns the FULL [32, 1000] fp32 output.
"""

import math

import numpy as np

import concourse.bass as bass
from concourse import bacc
import concourse.mybir as mybir
import concourse.tile as tile
from concourse.bass_utils import run_bass_kernel_spmd

F32 = mybir.dt.float32

# ---------------------------------------------------------------- config

class Cfg:
    def __init__(self, B=16, L=196, D=768, C=250, J=10, NB=2, NC=4,
                 fp=mybir.dt.float16, pb=4):
        self.B, self.L, self.D, self.C, self.J = B, L, D, C, J
        self.NB, self.NC = NB, NC          # mesh: batch shards x class shards
        self.fp = fp
        self.BL = B * L                    # rows of z per core (3136)
        self.KD = D // 128                 # k chunks (6)
        assert D % 128 == 0
        self.NZT = math.ceil(self.BL / 128)        # z row chunks (25)
        self.BLP = self.NZT * 128                  # padded rows (3200)
        self.NCJ = math.ceil(C * J / 128)          # cj chunks (20)
        self.CJP = self.NCJ * 128                  # padded cj (2560)
        assert self.CJP % J == 0
        self.CP = self.CJP // J                    # padded classes (256)
        assert self.CP % 128 == 0
        self.NCH = self.CP // 128                  # class halves (2)
        assert self.NCJ == self.NCH * J
        self.NBLK_W = 2 * L                        # psum block width (392)
        assert self.NBLK_W <= 512
        assert self.BL % self.NBLK_W == 0
        self.NBLK = self.BL // self.NBLK_W         # bl blocks (8)
        self.PB = min(pb, self.NBLK)               # blocks per psum tile (4)
        assert self.NBLK % self.PB == 0
        self.NG = self.NBLK // self.PB             # bl groups / zT halves (2)
        # zT half h holds z-chunks [t_lo[h], t_hi[h]] (inclusive); a chunk
        # straddling a group boundary is transposed into both halves.
        self.t_lo, self.t_hi = [], []
        w = self.PB * self.NBLK_W                  # cols per group (1568)
        for h in range(self.NG):
            self.t_lo.append((h * w) // 128)
            self.t_hi.append(min(self.NZT - 1,
                                 math.ceil((h + 1) * w / 128) - 1))


# ---------------------------------------------------------------- device IR

def build_program(cfg: Cfg):
    nc = bacc.Bacc("TRN2", target_bir_lowering=False, debug=False)
    fp = cfg.fp
    KD, NZT, CJP, NCJ, CP, NCH, B, J = (cfg.KD, cfg.NZT, cfg.CJP, cfg.NCJ,
                                        cfg.CP, cfg.NCH, cfg.B, cfg.J)
    NBW, PB, NG, L = cfg.NBLK_W, cfg.PB, cfg.NG, cfg.L
    D = cfg.D

    z = nc.dram_tensor("z", [NZT, 128, D], fp, kind="ExternalInput").ap()
    pt = nc.dram_tensor("pt", [D, CJP], fp, kind="ExternalInput").ap()
    rw = nc.dram_tensor("rw", [128, NCJ], F32, kind="ExternalInput").ap()
    s01 = nc.dram_tensor("s01", [NCJ, 128, 128], F32, kind="ExternalInput").ap()
    bias2 = nc.dram_tensor("bias2", [NCH, 128], F32, kind="ExternalInput").ap()
    out = nc.dram_tensor("out", [NCH, 128, B], F32, kind="ExternalOutput").ap()
    wscratch = nc.dram_tensor("wscratch", [1, CJP], F32).ap()

    AF = mybir.ActivationFunctionType
    with tile.TileContext(nc) as tc:
        with tc.tile_pool(name="singles", bufs=1) as singles:
            zTs = [singles.tile([128, KD, (cfg.t_hi[h] - cfg.t_lo[h] + 1) * 128],
                                fp, name=f"zT{h}", tag=f"zT{h}")
                   for h in range(NG)]
            pt_sb = singles.tile([128, KD, CJP], fp)
            s_sb = singles.tile([128, NCJ, 128], F32)
            wv = singles.tile([128, NCJ], F32)
            bias_sb = singles.tile([128, NCH], F32)
            pooled = singles.tile([128, NCJ, B], F32)
            ones_sb = singles.tile([128, 1], fp)
            rw_sb = singles.tile([128, NCJ], F32)
            ss_row = singles.tile([1, CJP], F32)
            ssv = singles.tile([128, NCJ], F32)
            lnv = singles.tile([128, NCJ], F32)
            rpv = singles.tile([128, NCJ], F32)
            ev = singles.tile([128, NCJ], F32)
            spv = singles.tile([128, NCJ], F32)
            out_sb = singles.tile([128, NCH, B], F32)

            # ---- input loads, all on SWDGE (gpsimd) so the sync HWDGE
            # ring carries only the xbar transposes.  pt is loaded per
            # k-chunk so the p^2 pipeline starts before the full load.
            for k in range(KD):
                nc.gpsimd.dma_start(
                    out=pt_sb[:, k, :],
                    in_=pt.rearrange("(k p) c -> p k c", p=128)[:, k, :])
            nc.gpsimd.dma_start(out=rw_sb, in_=rw)
            nc.gpsimd.dma_start(out=bias_sb, in_=bias2.rearrange("h p -> p h"))
            nc.gpsimd.dma_start(out=s_sb, in_=s01.rearrange("t p c -> p t c"))
            nc.vector.memset(ones_sb, 1.0)

            # ---- z: load all chunks, batched row-normalize, batched
            # xbar transpose into the two half buffers.
            zcs = []
            with (
                tc.tile_pool(name="zc", bufs=1) as zc_pool,
                tc.tile_pool(name="zn", bufs=6) as zn_pool,
                tc.tile_pool(name="sq", bufs=3) as sq_pool,
            ):
                ss_all = singles.tile([128, NZT], F32)
                nz_all = singles.tile([128, NZT], F32)
                rz_all = singles.tile([128, NZT], F32)
                for t in range(NZT):
                    zc = zc_pool.tile([128, D], fp, tag=f"zc{t}")
                    nc.gpsimd.dma_start(out=zc, in_=z[t])
                    zcs.append(zc)
                for t in range(NZT):
                    sq = sq_pool.tile([128, D], fp)
                    nc.scalar.activation(sq, zcs[t], AF.Square,
                                         accum_out=ss_all[:, t:t + 1])
                    # sqrt+recip batched over groups of chunks to keep
                    # the transpose pipeline fed with low op overhead
                    if t == NZT - 1 or t == cfg.t_hi[0]:
                        lo = 0 if t == cfg.t_hi[0] else cfg.t_hi[0] + 1
                        sl = slice(lo, t + 1)
                        # 1/sqrt(x) = exp(-0.5 * ln(x)); ACT exp/ln are
                        # ~2 ULP class splines, plenty for normalization
                        nc.scalar.activation(nz_all[:, sl], ss_all[:, sl],
                                             AF.Ln)
                        nc.scalar.activation(rz_all[:, sl], nz_all[:, sl],
                                             AF.Exp, scale=-0.5)
                for t in range(NZT):
                    zn = zn_pool.tile([128, D], fp)
                    nc.vector.tensor_scalar_mul(out=zn, in0=zcs[t],
                                                scalar1=rz_all[:, t:t + 1])
                    for h in range(NG):
                        if cfg.t_lo[h] <= t <= cfg.t_hi[h]:
                            lt = t - cfg.t_lo[h]
                            nc.sync.dma_start_transpose(
                                zTs[h][:, :, lt * 128:(lt + 1) * 128], zn)

            # ---- prototype column sumsq -> w' = softplus(rw) / ||p||
            with (
                tc.tile_pool(name="p2", bufs=KD) as p2_pool,
                tc.tile_pool(name="ss_ps", bufs=1, space="PSUM") as ss_pool,
            ):
                ss_ps = ss_pool.tile([1, CJP], F32)
                p2s = []
                for k in range(KD):
                    p2 = p2_pool.tile([128, CJP], fp)
                    nc.vector.tensor_mul(p2, pt_sb[:, k, :], pt_sb[:, k, :])
                    p2s.append(p2)
                for n0 in range(0, CJP, 512):
                    sl = slice(n0, min(n0 + 512, CJP))
                    for k in range(KD):
                        nc.tensor.matmul(ss_ps[0:1, sl], lhsT=ones_sb,
                                         rhs=p2s[k][:, sl],
                                         start=(k == 0), stop=(k == KD - 1))
                nc.scalar.activation(ss_row, ss_ps, AF.Copy)
            # scatter ss [1, CJP] -> [128, NCJ] (cj = t*128 + p) via DRAM
            nc.gpsimd.dma_start(out=wscratch, in_=ss_row)
            nc.gpsimd.dma_start(
                out=ssv, in_=wscratch.rearrange("a (t p) -> (a p) t", p=128))
            # rp = 1/||p|| = exp(-0.5*ln(ss))
            nc.scalar.activation(lnv, ssv, AF.Ln)
            nc.scalar.activation(rpv, lnv, AF.Exp, scale=-0.5)
            # softplus(rw) = ln(exp(rw) + 1), already in [128, NCJ] layout
            nc.scalar.activation(ev, rw_sb, AF.Exp)
            nc.scalar.activation(spv, ev, AF.Ln, bias=1.0)
            nc.vector.tensor_mul(wv, spv, rpv)
            for t in range(NCJ):
                nc.vector.tensor_scalar_mul(out=s_sb[:, t, :],
                                            in0=s_sb[:, t, :],
                                            scalar1=wv[:, t:t + 1])

            # ---- main: sim^T chunks + fused max-over-l, one bl-half at
            # a time (half h only needs zTs[h]); k outer within a psum
            # tile so the 4 consecutive matmuls share the stationary.
            with tc.tile_pool(name="mm", bufs=2, space="PSUM") as mm_pool:
                for h in range(NG):
                    for ci in range(NCJ):
                        ps = mm_pool.tile([128, PB * 512], F32)
                        for k in range(KD):
                            for blk in range(PB):
                                col0 = ((h * PB + blk) * NBW
                                        - cfg.t_lo[h] * 128)
                                nc.tensor.matmul(
                                    ps[:, blk * 512: blk * 512 + NBW],
                                    lhsT=pt_sb[:, k, ci * 128:(ci + 1) * 128],
                                    rhs=zTs[h][:, k, col0:col0 + NBW],
                                    start=(k == 0), stop=(k == KD - 1),
                                    skip_group_check=True)
                        view = ps.rearrange("p (blk c) -> p blk c", blk=PB)
                        view = view[:, :, 0:NBW]
                        view = view.rearrange("p blk (s l) -> p blk s l", l=L)
                        nc.vector.reduce_max(
                            out=pooled[:, ci, h * PB * 2:(h + 1) * PB * 2],
                            in_=view, axis=mybir.AxisListType.X)

            # ---- j-sum via block-diagonal selector matmul + bias
            with tc.tile_pool(name="ops", bufs=NCH, space="PSUM") as out_pool:
                for h in range(NCH):
                    op = out_pool.tile([128, B], F32)
                    for i, t in enumerate(range(h * J, (h + 1) * J)):
                        nc.tensor.matmul(op, lhsT=s_sb[:, t, :],
                                         rhs=pooled[:, t, :],
                                         start=(i == 0), stop=(i == J - 1))
                    nc.scalar.activation(out_sb[:, h, :], op, AF.Identity,
                                         bias=bias_sb[:, h:h + 1])
            nc.sync.dma_start(out=out.rearrange("h p b -> p h b"), in_=out_sb)
    return nc


# ---------------------------------------------------------------- host side

def make_core_inputs(cfg: Cfg, spatial, prototypes, raw_weights, bias, ib, ic):
    """Build the input map for core (ib, ic) from FULL fp32 inputs."""
    npfp = np.float16 if cfg.fp == mybir.dt.float16 else np.float32
    B, L, D, C, J = cfg.B, cfg.L, cfg.D, cfg.C, cfg.J

    zs = spatial[ib * B:(ib + 1) * B].reshape(cfg.BL, D)
    z = np.ones((cfg.BLP, D), dtype=npfp)
    z[:cfg.BL] = zs.astype(npfp)
    z = z.reshape(cfg.NZT, 128, D)

    ps = prototypes[ic * C:(ic + 1) * C].reshape(C * J, D)
    pfull = np.ones((cfg.CJP, D), dtype=npfp)
    pfull[:C * J] = ps.astype(npfp)
    pt = np.ascontiguousarray(pfull.T)                      # [D, CJP]

    rwf = np.zeros(cfg.CJP, dtype=np.float32)
    rwf[:C * J] = raw_weights[ic * C:(ic + 1) * C].reshape(-1)
    rw = np.ascontiguousarray(rwf.reshape(cfg.NCJ, 128).T)   # [128, NCJ]

    b2 = np.zeros((cfg.NCH, 128), dtype=np.float32)
    b2.reshape(-1)[:C] = bias[ic * C:(ic + 1) * C]

    return {"z": z, "pt": pt, "rw": rw, "b": b2}


def make_s01(cfg: Cfg):
    """[NCJ, 128, 128] 0/1 selector: chunk t row p -> local class column."""
    s = np.zeros((cfg.NCJ, 128, 128), dtype=np.float32)
    for t in range(cfg.NCJ):
        h = t // cfg.J
        cj = t * 128 + np.arange(128)
        c_local = cj // cfg.J - h * 128
        s[t, np.arange(128), c_local] = 1.0
    return s


_CACHE = {}


def _get_program(key="main"):
    if key not in _CACHE:
        cfg = Cfg()
        nc = build_program(cfg)
        nc.finalize()          # run bacc lowering (wait-splitting, reg alloc)
        _CACHE[key] = (cfg, nc)
    return _CACHE[key]


def _run(spatial, prototypes, raw_weights, bias, **spmd_kwargs):
    spatial = np.asarray(spatial, dtype=np.float32)
    prototypes = np.asarray(prototypes, dtype=np.float32)
    raw_weights = np.asarray(raw_weights, dtype=np.float32)
    bias = np.asarray(bias, dtype=np.float32)

    cfg, nc = _get_program()
    s01 = make_s01(cfg)
    in_maps = []
    for core in range(cfg.NB * cfg.NC):
        ib, ic = core // cfg.NC, core % cfg.NC
        m = make_core_inputs(cfg, spatial, prototypes, raw_weights, bias,
                             ib, ic)
        in_maps.append({"z": m["z"], "pt": m["pt"], "rw": m["rw"],
                        "s01": s01, "bias2": m["b"]})

    res = run_bass_kernel_spmd(nc, in_maps,
                               core_ids=list(range(cfg.NB * cfg.NC)),
                               **spmd_kwargs)

    b_full = cfg.NB * cfg.B
    c_full = cfg.NC * cfg.C
    outp = np.zeros((b_full, c_full), dtype=np.float32)
    for core in range(cfg.NB * cfg.NC):
        ib, ic = core // cfg.NC, core % cfg.NC
        o = res.results[core]["out"].reshape(cfg.CP, cfg.B)    # [c_pad, b]
        outp[ib * cfg.B:(ib + 1) * cfg.B,
             ic * cfg.C:(ic + 1) * cfg.C] = o[:cfg.C].T
    return outp, res


def kernel(spatial, prototypes, raw_weights, bias):
    outp, _ = _run(spatial, prototypes, raw_weights, bias)
    return outp
